# revision 21
# baseline (speedup 1.0000x reference)
"""Trainium2 Bass kernel for BlockUncertaintyTracker (segment_reduce).

Computes, per 4x4 block of a [16,1,2048,2048] image batch:
  - mean over the 16 block elements, averaged over batch
  - 0.9-quantile (= 0.5*(2nd largest + 3rd largest)), averaged over batch
  - EMA update of both stats, then broadcasts the ratio back to full shape.

Sharding: spatial over H across 8 cores (256 image rows / 64 block rows per
core). Every core sees all 16 batch elements for its rows, so no collectives
are needed; EMA buffer slices are contiguous per core.

Engine split per group of 2 batches (4 row-phase tiles R_r [128,2048] f32):
  - ScalarE: cast R_r -> fp16, and even/odd de-interleaves between merge
    levels so every DVE tensor_tensor runs contiguous step-1 fp16 (2x mode).
  - VectorE: vertical sorted-3 across the 4 row tiles, then two merge levels
    down to (2nd+3rd largest) per block.
  - TensorE: block sums via 16 strided-rhs f32 matmuls against a 0/1 matrix
    (exact f32 mean path) + batch accumulation of the quantile stat in PSUM.
  - Output: ratio computed in a row-duplicated [128,512] layout, expanded and
    written as 32 full-partition 1 MiB DMAs.
"""

import os

import numpy as np

# ---- problem constants (hardcoded; kernel.py must be self-contained) ----
B = 16          # batch
H = 2048
W = 2048
BS = 4          # block size
NCORES = 8
HS = H // NCORES            # 256 rows per core
NBH = HS // BS              # 64 block rows per core
NBW = W // BS               # 512 block cols
ROWS = B * HS               # 4096 rows in a per-core slab
NGROUPS = 8                 # groups per core; each = 2 batches x 256 rows
GB = B // NGROUPS           # 2 batches per group
DECAY = 0.99
ALPHA = 0.1
EPS = 1e-5
C_MEAN = (1.0 - DECAY) / (BS * BS * B)    # fold mean-over-16-elems and batch
C_QUANT = (1.0 - DECAY) * 0.5 / B         # fold 0.5*(m2+m3) and batch mean

_CACHE = {}


def _split_multi_waits(nc):
    """This walrus build encodes at most ONE sync wait per instruction.
    Tile attaches several. Hoist excess waits onto same-engine NOPs placed
    immediately before the owning instruction (same engine stream => same
    semantics)."""
    import concourse.mybir as mybir

    plans = []  # (inst_name, extra_waits)
    for f in nc.m.functions:
        for bb in f.blocks:
            for inst in bb.instructions:
                si = getattr(inst, "sync_info", None)
                waits = list(si.on_wait) if (si and si.on_wait) else []
                if len(waits) > 1:
                    si.on_wait = [waits[-1]]
                    plans.append((inst.name, waits[:-1]))

    if not plans:
        return

    nop_for = {}
    stray = set()
    for iname, extra in plans:
        nops = []
        for w in extra:
            nop = nc.engines[nc.inst_map[iname].engine].nop(nofuse=True).ins
            nop.sync_info = mybir.SyncInfo(on_wait=[w], on_update=[])
            nops.append(nop)
            stray.add(nop.name)
        nop_for[iname] = nops

    for f in nc.m.functions:
        for bb in f.blocks:
            out = []
            changed = False
            for inst in bb.instructions:
                if inst.name in stray:
                    changed = True
                    continue
                if inst.name in nop_for:
                    out.extend(nop_for[inst.name])
                    changed = True
                out.append(inst)
            if changed:
                bb.instructions = out


def _build():
    """Builds the single-core Bass program (SPMD across 8 cores)."""
    from contextlib import ExitStack

    import concourse.bass as bass
    import concourse.mybir as mybir
    import concourse.tile as tile

    f32 = mybir.dt.float32
    f16 = mybir.dt.float16
    MAX = mybir.AluOpType.max
    MIN = mybir.AluOpType.min
    MULT = mybir.AluOpType.mult
    ADD = mybir.AluOpType.add

    nc = bass.Bass("TRN2", target_bir_lowering=False, debug=False)

    x = nc.dram_tensor("x", [ROWS, W], f32, kind="ExternalInput").ap()
    ee = nc.dram_tensor("ee", [NBH, NBW], f32, kind="ExternalInput").ap()
    eq = nc.dram_tensor("eq", [NBH, NBW], f32, kind="ExternalInput").ap()
    # ones2[p, m] = (p % 64 == m // 2): batch-pair fold + row duplication
    ones2 = nc.dram_tensor("ones2", [128, 128], f32, kind="ExternalInput").ap()
    y = nc.dram_tensor("y", [ROWS, W], f32, kind="ExternalOutput").ap()

    # input: row = ((g*2 + b2)*64 + i)*4 + r; per (g, r): [128=(b2,i), 2048]
    xr = x.rearrange("(g b2 i r) w -> g r (b2 i) w", g=NGROUPS, b2=GB, i=NBH, r=BS)
    # output: row = b*256 + 4i + 2h + r2; per (b, h): [64, 2, 2048] = 128
    # outer steps zipped against the SBUF tile's 128 partitions (p = 2i + r2)
    y5 = y.rearrange("(b i h r2) w -> b h i r2 w", b=B, i=NBH, h=2, r2=2)

    with tile.TileContext(nc) as tc, ExitStack() as ctx:
        pool = ctx.enter_context(tc.tile_pool(name="work", bufs=1))
        ppool = ctx.enter_context(tc.tile_pool(name="acc", bufs=1, space="PSUM"))

        psum_s = ppool.tile([128, NBW], f32, tag="ps")
        psum_q = ppool.tile([128, NBW], f32, tag="pq")

        ones_sb = pool.tile([128, 128], f32, tag="ones")
        nc.sync.dma_start(ones_sb[:, :], ones2)
        ones16_sb = pool.tile([128, 128], f16, tag="ones16")
        nc.scalar.copy(ones16_sb[:, :], ones_sb[:, :])

        def tt(dst, a, bb, op):
            nc.vector.tensor_tensor(dst, a, bb, op)

        # EMA buffers + prep early: DVE is idle during the first loads anyway
        ee_sb = pool.tile([128, NBW], f32, tag="tail", bufs=5, name="ee_sb")
        nc.sync.dma_start(ee_sb[:, :], ee.unsqueeze(1).broadcast_to((NBH, 2, NBW)))
        eq_sb = pool.tile([128, NBW], f32, tag="tail", bufs=5, name="eq_sb")
        nc.sync.dma_start(eq_sb[:, :], eq.unsqueeze(1).broadcast_to((NBH, 2, NBW)))
        ee2 = pool.tile([128, NBW], f32, tag="tail", bufs=5, name="ee2")
        nc.vector.tensor_scalar(ee2[:, :], ee_sb[:, :], DECAY, EPS, MULT, ADD)
        eq2 = pool.tile([128, NBW], f32, tag="tail", bufs=5, name="eq2")
        nc.vector.tensor_scalar(eq2[:, :], eq_sb[:, :], DECAY, 0.0, MULT, ADD)

        rts_g = {}
        bts_g = {}
        planes_g = {}
        aouts_g = {}

        def emit_load(g):
            rts = []
            for r in range(BS):
                rt = pool.tile([128, W], f32, tag=f"r{r}", bufs=2, name=f"rt{r}_{g}")
                nc.sync.dma_start(rt[:, :], xr[g, r])
                rts.append(rt)
            rts_g[g] = rts

        def emit_sum(g):
            rts = rts_g[g]
            s01 = pool.tile([128, W], f32, tag="s01", bufs=1, name=f"s01_{g}")
            tt(s01[:, :], rts[0][:, :], rts[1][:, :], ADD)
            k0 = g * 12
            sv = s01.rearrange("p (j c) -> p j c", c=BS)
            for c in range(BS):
                nc.tensor.matmul(
                    psum_s[:, :], lhsT=ones_sb[:, :], rhs=sv[:, :, c],
                    start=(k0 + c == 0), stop=False,
                )
            for r in (2, 3):
                rv = rts[r].rearrange("p (j c) -> p j c", c=BS)
                for c in range(BS):
                    k = k0 + 4 + (r - 2) * BS + c
                    nc.tensor.matmul(
                        psum_s[:, :], lhsT=ones_sb[:, :], rhs=rv[:, :, c],
                        start=False, stop=(k == NGROUPS * 12 - 1),
                    )

        def emit_casts(g):
            bts = []
            for r in range(BS):
                bt = pool.tile([128, W], f16, tag=f"b{r}", bufs=2, name=f"bt{r}_{g}")
                nc.scalar.copy(bt[:, 0 : W // 2], rts_g[g][r][:, 0 : W // 2])
                nc.scalar.copy(bt[:, W // 2 : W], rts_g[g][r][:, W // 2 : W])
                bts.append(bt)
            bts_g[g] = bts

        def emit_vertical(g):
            b0, b1, b2_, b3 = bts_g[g]
            v1 = pool.tile([128, W], f16, tag="big", bufs=9, name=f"v1_{g}")
            tt(v1[:, :], b0[:, :], b1[:, :], MAX)
            w1v = pool.tile([128, W], f16, tag="big", bufs=9, name=f"w1v_{g}")
            tt(w1v[:, :], b0[:, :], b1[:, :], MIN)
            v2 = pool.tile([128, W], f16, tag="big", bufs=9, name=f"v2_{g}")
            tt(v2[:, :], b2_[:, :], b3[:, :], MAX)
            w2v = pool.tile([128, W], f16, tag="big", bufs=9, name=f"w2v_{g}")
            tt(w2v[:, :], b2_[:, :], b3[:, :], MIN)
            m = pool.tile([128, W], f16, tag="big", bufs=9, name=f"m_{g}")
            tt(m[:, :], v1[:, :], v2[:, :], MAX)
            t1 = pool.tile([128, W], f16, tag="big", bufs=9, name=f"t1_{g}")
            tt(t1[:, :], v1[:, :], v2[:, :], MIN)
            t2 = pool.tile([128, W], f16, tag="big", bufs=9, name=f"t2_{g}")
            tt(t2[:, :], w1v[:, :], w2v[:, :], MAX)
            s2 = pool.tile([128, W], f16, tag="big", bufs=9, name=f"s2_{g}")
            tt(s2[:, :], t1[:, :], t2[:, :], MAX)
            t3 = pool.tile([128, W], f16, tag="big", bufs=9, name=f"t3_{g}")
            tt(t3[:, :], t1[:, :], t2[:, :], MIN)
            planes_g[g] = (m, s2, t3)

        HW2 = W // 2

        def deint(src, w_out, tag, name):
            v = src.rearrange("p (j two) -> p j two", two=2)
            te = pool.tile([128, w_out], f16, tag=tag, bufs=7, name=name + "e")
            nc.scalar.copy(te[:, :], v[:, :, 0])
            to = pool.tile([128, w_out], f16, tag=tag, bufs=7, name=name + "o")
            nc.scalar.copy(to[:, :], v[:, :, 1])
            return te, to

        def emit_alevel(g):
            m, s2, t3 = planes_g[g]
            me, mo = deint(m, HW2, "eoa", f"m_{g}")
            s2e, s2o = deint(s2, HW2, "eoa", f"s2_{g}")
            t3e, t3o = deint(t3, HW2, "eoa", f"t3_{g}")
            p1 = pool.tile([128, HW2], f16, tag="mid", bufs=6, name=f"p1_{g}")
            tt(p1[:, :], me[:, :], mo[:, :], MAX)
            u1 = pool.tile([128, HW2], f16, tag="mid", bufs=6, name=f"u1_{g}")
            tt(u1[:, :], me[:, :], mo[:, :], MIN)
            u2 = pool.tile([128, HW2], f16, tag="mid", bufs=6, name=f"u2_{g}")
            tt(u2[:, :], s2e[:, :], s2o[:, :], MAX)
            p2 = pool.tile([128, HW2], f16, tag="mid", bufs=6, name=f"p2_{g}")
            tt(p2[:, :], u1[:, :], u2[:, :], MAX)
            w2 = pool.tile([128, HW2], f16, tag="mid", bufs=6, name=f"w2_{g}")
            tt(w2[:, :], me[:, :], s2o[:, :], MIN)
            w3 = pool.tile([128, HW2], f16, tag="mid", bufs=6, name=f"w3_{g}")
            tt(w3[:, :], s2e[:, :], mo[:, :], MIN)
            w4 = pool.tile([128, HW2], f16, tag="mid", bufs=6, name=f"w4_{g}")
            tt(w4[:, :], w2[:, :], w3[:, :], MAX)
            w1 = pool.tile([128, HW2], f16, tag="mid", bufs=6, name=f"w1_{g}")
            tt(w1[:, :], t3e[:, :], t3o[:, :], MAX)
            p3 = pool.tile([128, HW2], f16, tag="mid", bufs=6, name=f"p3_{g}")
            tt(p3[:, :], w1[:, :], w4[:, :], MAX)
            aouts_g[g] = (p1, p2, p3)

        def emit_blevel(g):
            p1, p2, p3 = aouts_g[g]
            p1e, p1o = deint(p1, NBW, "eob", f"p1_{g}")
            p2e, p2o = deint(p2, NBW, "eob", f"p2_{g}")
            p3e, p3o = deint(p3, NBW, "eob", f"p3_{g}")
            z1 = pool.tile([128, NBW], f16, tag="small", bufs=6, name=f"z1_{g}")
            tt(z1[:, :], p1e[:, :], p1o[:, :], MIN)
            z2 = pool.tile([128, NBW], f16, tag="small", bufs=6, name=f"z2_{g}")
            tt(z2[:, :], p2e[:, :], p2o[:, :], MAX)
            c2 = pool.tile([128, NBW], f16, tag="small", bufs=6, name=f"c2_{g}")
            tt(c2[:, :], z1[:, :], z2[:, :], MAX)
            z4 = pool.tile([128, NBW], f16, tag="small", bufs=6, name=f"z4_{g}")
            tt(z4[:, :], p1e[:, :], p2o[:, :], MIN)
            z5 = pool.tile([128, NBW], f16, tag="small", bufs=6, name=f"z5_{g}")
            tt(z5[:, :], p2e[:, :], p1o[:, :], MIN)
            z6 = pool.tile([128, NBW], f16, tag="small", bufs=6, name=f"z6_{g}")
            tt(z6[:, :], z4[:, :], z5[:, :], MAX)
            z3 = pool.tile([128, NBW], f16, tag="small", bufs=6, name=f"z3_{g}")
            tt(z3[:, :], p3e[:, :], p3o[:, :], MAX)
            c3 = pool.tile([128, NBW], f16, tag="small", bufs=6, name=f"c3_{g}")
            tt(c3[:, :], z3[:, :], z6[:, :], MAX)
            qs = pool.tile([128, NBW], f16, tag="qs", bufs=2, name=f"qs_{g}")
            tt(qs[:, :], c2[:, :], c3[:, :], ADD)
            nc.tensor.matmul(
                psum_q[:, :], lhsT=ones16_sb[:, :], rhs=qs[:, :],
                start=(g == 0), stop=(g == NGROUPS - 1),
            )

        den = [None]
        rec = [None]
        for g in range(NGROUPS):
            emit_load(g)
            emit_sum(g)
            if g == NGROUPS - 1:
                # psum_s is final here; overlap den + reciprocal with the
                # last group's sort compute
                dn = pool.tile([128, NBW], f32, tag="tail", bufs=5, name="den")
                nc.vector.scalar_tensor_tensor(
                    dn[:, :], psum_s[:, :], C_MEAN, ee2[:, :], op0=MULT, op1=ADD
                )
                rc = pool.tile([128, NBW], f32, tag="tail", bufs=5, name="rec")
                nc.vector.reciprocal(rc[:, :], dn[:, :])
                den[0] = dn
                rec[0] = rc
            emit_casts(g)
            emit_vertical(g)
            emit_alevel(g)
            emit_blevel(g)

        # ---- tail: num + ratio + broadcast (den/rec were done in-loop) ----
        num = pool.tile([128, NBW], f32, tag="tail", bufs=5, name="num")
        nc.vector.scalar_tensor_tensor(
            num[:, :], psum_q[:, :], C_QUANT, eq2[:, :], op0=MULT, op1=ADD
        )
        u = pool.tile([128, NBW], f32, tag="tail", bufs=5, name="u")
        nc.vector.tensor_tensor(u[:, :], num[:, :], rec[0][:, :], MULT)

        # expand x4 along columns: u4[p, j*4 + c] = u[p, j]; split DVE/ACT
        u4 = pool.tile([128, W], f32, tag="u4")
        u4v = u4.rearrange("p (j c) -> p j c", c=BS)
        nc.vector.tensor_copy(u4v[:, :, 0], u[:, :])
        nc.scalar.copy(u4v[:, :, 1], u[:, :])
        nc.vector.tensor_copy(u4v[:, :, 2], u[:, :])
        nc.scalar.copy(u4v[:, :, 3], u[:, :])

        # 32 full-partition writes: batch x row-pair-half
        for b in range(B):
            for h in range(2):
                nc.sync.dma_start(y5[b, h], u4[:, :])

    _split_multi_waits(nc)
    return nc


def _get_nc():
    if "nc" not in _CACHE:
        _CACHE["nc"] = _build()
    return _CACHE["nc"]


def kernel(current_errors, ema_errors, ema_quantile):
    from concourse.bass_utils import run_bass_kernel_spmd

    x = np.asarray(current_errors, dtype=np.float32).reshape(B, H, W)
    ee = np.asarray(ema_errors, dtype=np.float32).reshape(H // BS, W // BS)
    eq = np.asarray(ema_quantile, dtype=np.float32).reshape(H // BS, W // BS)

    # ones2[p, m] == 1 iff p % 64 == m // 2
    ones2 = np.zeros((128, 128), dtype=np.float32)
    p = np.arange(128)
    ones2[p, (p % NBH) * 2] = 1.0
    ones2[p, (p % NBH) * 2 + 1] = 1.0

    in_maps = []
    for k in range(NCORES):
        xs = np.ascontiguousarray(x[:, k * HS : (k + 1) * HS, :]).reshape(ROWS, W)
        ees = np.ascontiguousarray(ee[k * NBH : (k + 1) * NBH, :])
        eqs = np.ascontiguousarray(eq[k * NBH : (k + 1) * NBH, :])
        in_maps.append({"x": xs, "ee": ees, "eq": eqs, "ones2": ones2})

    nc = _get_nc()
    trace = bool(int(os.environ.get("KERNEL_TRACE", "0")))
    try:
        res = run_bass_kernel_spmd(
            nc, in_maps, core_ids=list(range(NCORES)), trace=trace
        )
    except Exception:
        # transient device state (e.g. NRT_EXEC_UNIT_UNRECOVERABLE) — retry once
        res = run_bass_kernel_spmd(
            nc, in_maps, core_ids=list(range(NCORES)), trace=trace
        )
    _CACHE["last_results"] = res

    out = np.empty((B, 1, H, W), dtype=np.float32)
    for k in range(NCORES):
        out[:, 0, k * HS : (k + 1) * HS, :] = res.results[k]["y"].reshape(B, HS, W)
    return out


# revision 22
# speedup vs baseline: 1.0715x; 1.0715x over previous
"""Trainium2 Bass kernel for BlockUncertaintyTracker (segment_reduce).

Computes, per 4x4 block of a [16,1,2048,2048] image batch:
  - mean over the 16 block elements, averaged over batch
  - 0.9-quantile (= 0.5*(2nd largest + 3rd largest)), averaged over batch
  - EMA update of both stats, then broadcasts the ratio back to full shape.

Sharding: spatial over H across 8 cores (256 image rows / 64 block rows per
core). Every core sees all 16 batch elements for its rows, so no collectives
are needed; EMA buffer slices are contiguous per core.

Engine split per group of 2 batches (4 row-phase tiles R_r [128,2048] f32):
  - ScalarE: cast R_r -> fp16, and even/odd de-interleaves between merge
    levels so every DVE tensor_tensor runs contiguous step-1 fp16 (2x mode).
  - VectorE: vertical sorted-3 across the 4 row tiles, then two merge levels
    down to (2nd+3rd largest) per block.
  - TensorE: block sums via 16 strided-rhs f32 matmuls against a 0/1 matrix
    (exact f32 mean path) + batch accumulation of the quantile stat in PSUM.
  - Output: ratio computed in a row-duplicated [128,512] layout, expanded and
    written as 32 full-partition 1 MiB DMAs.
"""

import os

import numpy as np

# ---- problem constants (hardcoded; kernel.py must be self-contained) ----
B = 16          # batch
H = 2048
W = 2048
BS = 4          # block size
NCORES = 8
HS = H // NCORES            # 256 rows per core
NBH = HS // BS              # 64 block rows per core
NBW = W // BS               # 512 block cols
ROWS = B * HS               # 4096 rows in a per-core slab
NGROUPS = 8                 # groups per core; each = 2 batches x 256 rows
GB = B // NGROUPS           # 2 batches per group
DECAY = 0.99
ALPHA = 0.1
EPS = 1e-5
C_MEAN = (1.0 - DECAY) / (BS * BS * B)    # fold mean-over-16-elems and batch
C_QUANT = (1.0 - DECAY) * 0.5 / B         # fold 0.5*(m2+m3) and batch mean

_CACHE = {}


def _split_multi_waits(nc):
    """This walrus build encodes at most ONE sync wait per instruction.
    Tile attaches several. Hoist excess waits onto same-engine NOPs placed
    immediately before the owning instruction (same engine stream => same
    semantics)."""
    import concourse.mybir as mybir

    plans = []  # (inst_name, extra_waits)
    for f in nc.m.functions:
        for bb in f.blocks:
            for inst in bb.instructions:
                si = getattr(inst, "sync_info", None)
                waits = list(si.on_wait) if (si and si.on_wait) else []
                if len(waits) > 1:
                    si.on_wait = [waits[-1]]
                    plans.append((inst.name, waits[:-1]))

    if not plans:
        return

    nop_for = {}
    stray = set()
    for iname, extra in plans:
        nops = []
        for w in extra:
            nop = nc.engines[nc.inst_map[iname].engine].nop(nofuse=True).ins
            nop.sync_info = mybir.SyncInfo(on_wait=[w], on_update=[])
            nops.append(nop)
            stray.add(nop.name)
        nop_for[iname] = nops

    for f in nc.m.functions:
        for bb in f.blocks:
            out = []
            changed = False
            for inst in bb.instructions:
                if inst.name in stray:
                    changed = True
                    continue
                if inst.name in nop_for:
                    out.extend(nop_for[inst.name])
                    changed = True
                out.append(inst)
            if changed:
                bb.instructions = out


def _build():
    """Builds the single-core Bass program (SPMD across 8 cores)."""
    from contextlib import ExitStack

    import concourse.bass as bass
    import concourse.mybir as mybir
    import concourse.tile as tile

    f32 = mybir.dt.float32
    f16 = mybir.dt.float16
    MAX = mybir.AluOpType.max
    MIN = mybir.AluOpType.min
    MULT = mybir.AluOpType.mult
    ADD = mybir.AluOpType.add

    nc = bass.Bass("TRN2", target_bir_lowering=False, debug=False)

    x = nc.dram_tensor("x", [ROWS, W], f32, kind="ExternalInput").ap()
    ee = nc.dram_tensor("ee", [NBH, NBW], f32, kind="ExternalInput").ap()
    eq = nc.dram_tensor("eq", [NBH, NBW], f32, kind="ExternalInput").ap()
    # ones2[p, m] = (p % 64 == m // 2): batch-pair fold + row duplication
    ones2 = nc.dram_tensor("ones2", [128, 128], f32, kind="ExternalInput").ap()
    y = nc.dram_tensor("y", [ROWS, W], f32, kind="ExternalOutput").ap()

    # input: row = ((g*2 + b2)*64 + i)*4 + r; per (g, r): [128=(b2,i), 2048]
    xr = x.rearrange("(g b2 i r) w -> g r (b2 i) w", g=NGROUPS, b2=GB, i=NBH, r=BS)
    # output: row = b*256 + 4i + 2h + r2; per (b, h): [64, 2, 2048] = 128
    # outer steps zipped against the SBUF tile's 128 partitions (p = 2i + r2)
    y5 = y.rearrange("(b i h r2) w -> b h i r2 w", b=B, i=NBH, h=2, r2=2)

    with tile.TileContext(nc) as tc, ExitStack() as ctx:
        pool = ctx.enter_context(tc.tile_pool(name="work", bufs=1))
        ppool = ctx.enter_context(tc.tile_pool(name="acc", bufs=1, space="PSUM"))

        psum_s = ppool.tile([128, NBW], f32, tag="ps")
        psum_q = ppool.tile([128, NBW], f32, tag="pq")

        ones_sb = pool.tile([128, 128], f32, tag="ones")
        nc.sync.dma_start(ones_sb[:, :], ones2)
        ones16_sb = pool.tile([128, 128], f16, tag="ones16")
        nc.scalar.copy(ones16_sb[:, :], ones_sb[:, :])

        def tt(dst, a, bb, op):
            nc.vector.tensor_tensor(dst, a, bb, op)

        rts_g = {}
        bts_g = {}
        planes_g = {}
        aouts_g = {}

        def emit_load(g):
            rts = []
            for r in range(BS):
                rt = pool.tile([128, W], f32, tag=f"r{r}", bufs=2, name=f"rt{r}_{g}")
                nc.sync.dma_start(rt[:, :], xr[g, r])
                rts.append(rt)
            rts_g[g] = rts

        def emit_sum(g):
            rts = rts_g[g]
            s01 = pool.tile([128, W], f32, tag="s01", bufs=1, name=f"s01_{g}")
            tt(s01[:, :], rts[0][:, :], rts[1][:, :], ADD)
            k0 = g * 12
            sv = s01.rearrange("p (j c) -> p j c", c=BS)
            for c in range(BS):
                nc.tensor.matmul(
                    psum_s[:, :], lhsT=ones_sb[:, :], rhs=sv[:, :, c],
                    start=(k0 + c == 0), stop=False,
                )
            for r in (2, 3):
                rv = rts[r].rearrange("p (j c) -> p j c", c=BS)
                for c in range(BS):
                    k = k0 + 4 + (r - 2) * BS + c
                    nc.tensor.matmul(
                        psum_s[:, :], lhsT=ones_sb[:, :], rhs=rv[:, :, c],
                        start=False, stop=(k == NGROUPS * 12 - 1),
                    )

        def emit_casts(g):
            bts = []
            for r in range(BS):
                bt = pool.tile([128, W], f16, tag=f"b{r}", bufs=2, name=f"bt{r}_{g}")
                nc.scalar.copy(bt[:, 0 : W // 2], rts_g[g][r][:, 0 : W // 2])
                nc.scalar.copy(bt[:, W // 2 : W], rts_g[g][r][:, W // 2 : W])
                bts.append(bt)
            bts_g[g] = bts

        def emit_vertical(g):
            b0, b1, b2_, b3 = bts_g[g]
            v1 = pool.tile([128, W], f16, tag="big", bufs=9, name=f"v1_{g}")
            tt(v1[:, :], b0[:, :], b1[:, :], MAX)
            w1v = pool.tile([128, W], f16, tag="big", bufs=9, name=f"w1v_{g}")
            tt(w1v[:, :], b0[:, :], b1[:, :], MIN)
            v2 = pool.tile([128, W], f16, tag="big", bufs=9, name=f"v2_{g}")
            tt(v2[:, :], b2_[:, :], b3[:, :], MAX)
            w2v = pool.tile([128, W], f16, tag="big", bufs=9, name=f"w2v_{g}")
            tt(w2v[:, :], b2_[:, :], b3[:, :], MIN)
            m = pool.tile([128, W], f16, tag="big", bufs=9, name=f"m_{g}")
            tt(m[:, :], v1[:, :], v2[:, :], MAX)
            t1 = pool.tile([128, W], f16, tag="big", bufs=9, name=f"t1_{g}")
            tt(t1[:, :], v1[:, :], v2[:, :], MIN)
            t2 = pool.tile([128, W], f16, tag="big", bufs=9, name=f"t2_{g}")
            tt(t2[:, :], w1v[:, :], w2v[:, :], MAX)
            s2 = pool.tile([128, W], f16, tag="big", bufs=9, name=f"s2_{g}")
            tt(s2[:, :], t1[:, :], t2[:, :], MAX)
            t3 = pool.tile([128, W], f16, tag="big", bufs=9, name=f"t3_{g}")
            tt(t3[:, :], t1[:, :], t2[:, :], MIN)
            planes_g[g] = (m, s2, t3)

        HW2 = W // 2

        def deint(src, w_out, tag, name):
            v = src.rearrange("p (j two) -> p j two", two=2)
            te = pool.tile([128, w_out], f16, tag=tag, bufs=7, name=name + "e")
            nc.scalar.copy(te[:, :], v[:, :, 0])
            to = pool.tile([128, w_out], f16, tag=tag, bufs=7, name=name + "o")
            nc.scalar.copy(to[:, :], v[:, :, 1])
            return te, to

        def emit_alevel(g):
            m, s2, t3 = planes_g[g]
            me, mo = deint(m, HW2, "eoa", f"m_{g}")
            s2e, s2o = deint(s2, HW2, "eoa", f"s2_{g}")
            t3e, t3o = deint(t3, HW2, "eoa", f"t3_{g}")
            p1 = pool.tile([128, HW2], f16, tag="mid", bufs=6, name=f"p1_{g}")
            tt(p1[:, :], me[:, :], mo[:, :], MAX)
            u1 = pool.tile([128, HW2], f16, tag="mid", bufs=6, name=f"u1_{g}")
            tt(u1[:, :], me[:, :], mo[:, :], MIN)
            u2 = pool.tile([128, HW2], f16, tag="mid", bufs=6, name=f"u2_{g}")
            tt(u2[:, :], s2e[:, :], s2o[:, :], MAX)
            p2 = pool.tile([128, HW2], f16, tag="mid", bufs=6, name=f"p2_{g}")
            tt(p2[:, :], u1[:, :], u2[:, :], MAX)
            w2 = pool.tile([128, HW2], f16, tag="mid", bufs=6, name=f"w2_{g}")
            tt(w2[:, :], me[:, :], s2o[:, :], MIN)
            w3 = pool.tile([128, HW2], f16, tag="mid", bufs=6, name=f"w3_{g}")
            tt(w3[:, :], s2e[:, :], mo[:, :], MIN)
            w4 = pool.tile([128, HW2], f16, tag="mid", bufs=6, name=f"w4_{g}")
            tt(w4[:, :], w2[:, :], w3[:, :], MAX)
            w1 = pool.tile([128, HW2], f16, tag="mid", bufs=6, name=f"w1_{g}")
            tt(w1[:, :], t3e[:, :], t3o[:, :], MAX)
            p3 = pool.tile([128, HW2], f16, tag="mid", bufs=6, name=f"p3_{g}")
            tt(p3[:, :], w1[:, :], w4[:, :], MAX)
            aouts_g[g] = (p1, p2, p3)

        def emit_blevel(g):
            p1, p2, p3 = aouts_g[g]
            p1e, p1o = deint(p1, NBW, "eob", f"p1_{g}")
            p2e, p2o = deint(p2, NBW, "eob", f"p2_{g}")
            p3e, p3o = deint(p3, NBW, "eob", f"p3_{g}")
            z1 = pool.tile([128, NBW], f16, tag="small", bufs=6, name=f"z1_{g}")
            tt(z1[:, :], p1e[:, :], p1o[:, :], MIN)
            z2 = pool.tile([128, NBW], f16, tag="small", bufs=6, name=f"z2_{g}")
            tt(z2[:, :], p2e[:, :], p2o[:, :], MAX)
            c2 = pool.tile([128, NBW], f16, tag="small", bufs=6, name=f"c2_{g}")
            tt(c2[:, :], z1[:, :], z2[:, :], MAX)
            z4 = pool.tile([128, NBW], f16, tag="small", bufs=6, name=f"z4_{g}")
            tt(z4[:, :], p1e[:, :], p2o[:, :], MIN)
            z5 = pool.tile([128, NBW], f16, tag="small", bufs=6, name=f"z5_{g}")
            tt(z5[:, :], p2e[:, :], p1o[:, :], MIN)
            z6 = pool.tile([128, NBW], f16, tag="small", bufs=6, name=f"z6_{g}")
            tt(z6[:, :], z4[:, :], z5[:, :], MAX)
            z3 = pool.tile([128, NBW], f16, tag="small", bufs=6, name=f"z3_{g}")
            tt(z3[:, :], p3e[:, :], p3o[:, :], MAX)
            c3 = pool.tile([128, NBW], f16, tag="small", bufs=6, name=f"c3_{g}")
            tt(c3[:, :], z3[:, :], z6[:, :], MAX)
            qs = pool.tile([128, NBW], f16, tag="qs", bufs=2, name=f"qs_{g}")
            tt(qs[:, :], c2[:, :], c3[:, :], ADD)
            nc.tensor.matmul(
                psum_q[:, :], lhsT=ones16_sb[:, :], rhs=qs[:, :],
                start=(g == 0), stop=(g == NGROUPS - 1),
            )

        for g in range(NGROUPS):
            emit_load(g)
            emit_sum(g)
            emit_casts(g)
            emit_vertical(g)
            emit_alevel(g)
            emit_blevel(g)

        # ---- tail: EMA update + ratio + broadcast (row-duplicated layout) ----
        ee_sb = pool.tile([128, NBW], f32, tag="tail", bufs=5, name="ee_sb")
        nc.sync.dma_start(ee_sb[:, :], ee.unsqueeze(1).broadcast_to((NBH, 2, NBW)))
        eq_sb = pool.tile([128, NBW], f32, tag="tail", bufs=5, name="eq_sb")
        nc.sync.dma_start(eq_sb[:, :], eq.unsqueeze(1).broadcast_to((NBH, 2, NBW)))

        ee2 = pool.tile([128, NBW], f32, tag="tail", bufs=5, name="ee2")
        nc.vector.tensor_scalar(ee2[:, :], ee_sb[:, :], DECAY, EPS, MULT, ADD)
        eq2 = pool.tile([128, NBW], f32, tag="tail", bufs=5, name="eq2")
        nc.vector.tensor_scalar(eq2[:, :], eq_sb[:, :], DECAY, 0.0, MULT, ADD)

        den = pool.tile([128, NBW], f32, tag="tail", bufs=5, name="den")
        nc.vector.scalar_tensor_tensor(
            den[:, :], psum_s[:, :], C_MEAN, ee2[:, :], op0=MULT, op1=ADD
        )
        num = pool.tile([128, NBW], f32, tag="tail", bufs=5, name="num")
        nc.vector.scalar_tensor_tensor(
            num[:, :], psum_q[:, :], C_QUANT, eq2[:, :], op0=MULT, op1=ADD
        )
        rec = pool.tile([128, NBW], f32, tag="tail", bufs=5, name="rec")
        nc.vector.reciprocal(rec[:, :], den[:, :])
        u = pool.tile([128, NBW], f32, tag="tail", bufs=5, name="u")
        nc.vector.tensor_tensor(u[:, :], num[:, :], rec[:, :], MULT)

        # expand x4 along columns: u4[p, j*4 + c] = u[p, j]
        u4 = pool.tile([128, W], f32, tag="u4")
        u4v = u4.rearrange("p (j c) -> p j c", c=BS)
        for c in range(BS):
            nc.vector.tensor_copy(u4v[:, :, c], u[:, :])

        # 32 full-partition writes: batch x row-pair-half
        for b in range(B):
            for h in range(2):
                nc.sync.dma_start(y5[b, h], u4[:, :])

    _split_multi_waits(nc)
    return nc


def _get_nc():
    if "nc" not in _CACHE:
        _CACHE["nc"] = _build()
    return _CACHE["nc"]


def kernel(current_errors, ema_errors, ema_quantile):
    from concourse.bass_utils import run_bass_kernel_spmd

    x = np.asarray(current_errors, dtype=np.float32).reshape(B, H, W)
    ee = np.asarray(ema_errors, dtype=np.float32).reshape(H // BS, W // BS)
    eq = np.asarray(ema_quantile, dtype=np.float32).reshape(H // BS, W // BS)

    # ones2[p, m] == 1 iff p % 64 == m // 2
    ones2 = np.zeros((128, 128), dtype=np.float32)
    p = np.arange(128)
    ones2[p, (p % NBH) * 2] = 1.0
    ones2[p, (p % NBH) * 2 + 1] = 1.0

    in_maps = []
    for k in range(NCORES):
        xs = np.ascontiguousarray(x[:, k * HS : (k + 1) * HS, :]).reshape(ROWS, W)
        ees = np.ascontiguousarray(ee[k * NBH : (k + 1) * NBH, :])
        eqs = np.ascontiguousarray(eq[k * NBH : (k + 1) * NBH, :])
        in_maps.append({"x": xs, "ee": ees, "eq": eqs, "ones2": ones2})

    nc = _get_nc()
    trace = bool(int(os.environ.get("KERNEL_TRACE", "0")))
    try:
        res = run_bass_kernel_spmd(
            nc, in_maps, core_ids=list(range(NCORES)), trace=trace
        )
    except Exception:
        # transient device state (e.g. NRT_EXEC_UNIT_UNRECOVERABLE) — retry once
        res = run_bass_kernel_spmd(
            nc, in_maps, core_ids=list(range(NCORES)), trace=trace
        )
    _CACHE["last_results"] = res

    out = np.empty((B, 1, H, W), dtype=np.float32)
    for k in range(NCORES):
        out[:, 0, k * HS : (k + 1) * HS, :] = res.results[k]["y"].reshape(B, HS, W)
    return out
